# revision 63
# baseline (speedup 1.0000x reference)
"""3x3 median filter (reflect padding) on Trainium2, data-parallel over batch.

Input:  image [16, 3, 512, 512] f32
Output: same shape; out[b,c,y,x] = median of the 3x3 window around (y,x),
        reflect padding.

Sharding: batch dim split across 8 NeuronCores (2 images per core), SPMD.

Compute runs in bf16 (rel err ~4e-3, within tolerance). The key TRN2 fact:
VectorE TENSOR_TENSOR runs at 2 elem/cycle (2x_1P mode) only for 16-bit
dtypes with innermost stride +-1 AND 4-byte-aligned streams; any stride-2
or odd-element-shifted operand falls back to 1 elem/cycle. The horizontal
median stage needs column-neighbor access, so:

Host prep: per-core input is staged to [H+2, BPC, C, 2, W/2] bf16 (both
images merged into each padded row so every DMA access pattern stays
3-dim) with even/odd columns DEINTERLEAVED (E plane = cols 0,2,...,
O plane = cols 1,3,...) and the two vertical reflect rows pre-staged.
Every horizontal op then reads aligned plane pairs, and the only shifted
(odd-offset) reads are done by the otherwise-idle ScalarE as clamped
copies into aligned scratch; every VectorE op runs at 2x.

Per-core algorithm (separable exact median with stride-2 vertical pair
sharing, per output pixel amortized: 5 vertical + 2 pair + 4 final +
4 med3 = 15 VectorE min/max elem-ops). Each SBUF partition owns a ROW
PAIR (padded rows r0+2p+1, r0+2p+2) shared by the two output rows
r0+2p (third row below) and r0+2p+1 (third row above), so the t1/t2
sort ops run once per pair and each padded row is DMA'd twice, not 3x.
4 steps of (one image, 256-row band); row parity rp lives on the free
axis; all TENSOR_TENSOR at 2x mode:
  1. Load pairT [128,2,C,2,Wh] + both third rows th [128,2,C,2,Wh]
     (2 DMAs; partition stride 2 rows, th's rp-dim stride 3 rows).
  2. t1/t2 = min/max of the pair (2 TT, FD=1536), then both parities'
     third-row merges stacked per op via stride-0 broadcast of t1/t2
     over rp: m, hi, lo, md (4 TT, FD=3072).
  3. Horizontal pairs on E/O column planes: melo,mxmd = max over
     (lo,md); mnmd,mehi = min over (md,hi) (2 stacked TT, FD=3072)
  4. ScalarE: sEO[0][k] = E-plane of {lo,md,hi} shifted left, clamped at
     the edge (for odd output cols); sEO[1][k] = O-plane shifted right,
     clamped (for even cols). The clamps reproduce the horizontal
     reflect boundary columns exactly (median of a {c,c',c} window
     degenerates to the pair reduction), so there is no boundary pass.
     Copies run per plane, ordered lo->md->hi, in the VectorE shadow.
  5. Finals, both column parities fused per op by broadcasting the
     shared pair operand over the parity dim with a stride-0 AP (4 TT,
     FD=3072):
       odd  col 2j+1: X=max(melo[j],loE[j+1]) Z=min(mehi[j],hiE[j+1])
                      Y=max(mnmd[j],min(mxmd[j],mdE[j+1]))
       even col 2j:   same with the single taken from O[j-1]
  6. median = med3(X, Y, Z) (4 TT, FD=3072); DMA out E/O planes per rp
     (the last step runs med3+DMA per rp to shorten the drain tail).

Measured on HW: 234.8 us (f32 1x baseline) -> 119.4 us. VectorE is the
bottleneck at ~97% busy; its TENSOR_TENSOR floor for this op count is
~106.4 us, plus ~7 us fixed engine preamble, ~5 us initial DMA fill and
~5.5 us drain tail.
"""

import sys

sys.path.insert(0, "/opt/trn_rl_repo")

import numpy as np

_COMPILED = {}

B, C, H, W = 16, 3, 512, 512
NCORES = 8
BPC = B // NCORES  # batches per core (stacked on the free axis)
RT = 128           # output rows per step
NRT = H // RT      # steps (each covers all BPC batches)
HP = H + 2         # padded rows on device
Wh = W // 2        # half width (E/O plane width)
SR = BPC * C * W   # padded-row stride (elements) in device layout
                   # [HP, BPC, C, 2, Wh] -- both batches live in one row


def _legalize_waits(nc, mybir):
    """Hoist excess sync-waits into a preceding same-engine EventSemaphore.
    The TRN2 ISA allows 1 sync-wait on compute instructions (2 on DMACopy;
    EventSemaphore allows several) but Tile's scheduler can emit more; a
    wait-only instruction earlier in the same engine's program order is
    semantically identical."""
    limits = {"InstEventSemaphore": 2}
    n_hoisted = 0
    for f in nc.m.functions:
        for bb in f.blocks:
            il = bb.instructions
            idx = 0
            while idx < len(il):
                i = il[idx]
                si = i.sync_info
                lim = limits.get(type(i).__name__, 1)
                if si is not None and si.on_wait and len(si.on_wait) > lim:
                    waits = list(si.on_wait)
                    keep, excess = waits[:lim], waits[lim:]
                    hoists = []
                    for j in range(0, len(excess), 2):
                        h = mybir.InstEventSemaphore(
                            name=f"hoistw_{n_hoisted}", ins=[], outs=[])
                        n_hoisted += 1
                        h.engine = i.engine
                        h.sync_info = mybir.SyncInfo(
                            on_wait=excess[j:j + 2], on_update=[])
                        hoists.append(h)
                    i.sync_info = mybir.SyncInfo(
                        on_wait=keep, on_update=si.on_update)
                    for k, h in enumerate(hoists):
                        il.insert(idx + k, h)
                    idx += len(hoists)
                idx += 1
    return n_hoisted


def _build_nc():
    from concourse import bass
    import concourse.mybir as mybir
    from concourse.tile import TileContext

    bf16 = mybir.dt.bfloat16
    MIN = mybir.AluOpType.min
    MAX = mybir.AluOpType.max
    AP = bass.AP

    nc = bass.Bass()
    img = nc.dram_tensor("image", [HP, BPC, C, 2, Wh], bf16,
                         kind="ExternalInput")
    out = nc.dram_tensor("out", [H, BPC, C, 2, Wh], bf16,
                         kind="ExternalOutput")

    SRB = C * W  # per-batch chunk of a padded row (1536)
    RB = 2 * RT  # output rows per step (each partition owns a ROW PAIR)
    steps = [(b, band) for b in range(BPC) for band in range(H // RB)]
    with TileContext(nc) as tc:
        with tc.tile_pool(name="p", bufs=2) as pool:
            for it, (b, band) in enumerate(steps):
                r0 = band * RB
                # ---- stride-2 vertical pair sharing: partition p owns the
                # row PAIR (padded rows r0+2p+1, r0+2p+2), which is shared
                # by output rows r0+2p (third = padded r0+2p) and r0+2p+1
                # (third = padded r0+2p+3). Each padded row is loaded twice
                # (not 3x) and the t1/t2 sort ops run once per pair.
                pairT = pool.tile([RT, 2, C, 2, Wh], bf16, tag="pair",
                                  bufs=3)
                # lmh slices: 0=lo 1=md 2=hi 3=m(scratch) 4=th(DMA dest);
                # keeping the third rows and m inside lmh lets BOTH merge
                # chains stack into single double-FD ops (see below).
                lmh = pool.tile([RT, 5, 2, C, 2, Wh], bf16, tag="lmh",
                                bufs=2)
                nc.sync.dma_start(out=pairT[:], in_=AP(
                    img, (r0 + 1) * SR + b * SRB,
                    [[2 * SR, RT], [SR, 2], [1, SRB]]))
                # both third rows in one DMA: rp=0 -> row r0+2p (below),
                # rp=1 -> row r0+2p+3 (above); rp-dim stride = 3 rows
                nc.sync.dma_start(out=lmh[:, 4], in_=AP(
                    img, r0 * SR + b * SRB,
                    [[2 * SR, RT], [3 * SR, 2], [1, SRB]]))

                def bcast2(h):
                    # insert a stride-0 broadcast dim after the partition dim
                    return AP(h.tensor, h.offset,
                              [list(h.ap[0])] + [[0, 2]] +
                              [list(q) for q in h.ap[1:]])

                def stk2(h, stride):
                    # stack dim of 2 (given stride) after the partition dim
                    return AP(h.tensor, h.offset,
                              [list(h.ap[0])] + [[stride, 2]] +
                              [list(q) for q in h.ap[1:]])

                def stkb(h, stride):
                    # stack dim of 2 plus an rp-broadcast dim (stride 0)
                    return AP(h.tensor, h.offset,
                              [list(h.ap[0])] + [[stride, 2], [0, 2]] +
                              [list(q) for q in h.ap[1:]])

                # ---- vertical sort3, 4 ops: t1/t2 once per pair (FD=1536);
                # then BOTH parities x BOTH merge chains stacked (FD=6144):
                #   [lo ; m]  = MIN([t1 ; t2] bcast over rp, [th ; th])
                #     (lo = min(t1, th) == min(t1, m) since t1 <= t2)
                #   [md ; hi] = MAX([t1 ; t2] bcast over rp, [m ; th])
                t12 = pool.tile([RT, 2, C, 2, Wh], bf16, tag="t12", bufs=1)
                t1, t2 = t12[:, 0], t12[:, 1]
                nc.vector.tensor_tensor(t1, pairT[:, 0], pairT[:, 1], MIN)
                nc.vector.tensor_tensor(t2, pairT[:, 0], pairT[:, 1], MAX)
                LS = 2 * C * 2 * Wh  # lmh slice stride (3072)
                th = lmh[:, 4]
                nc.vector.tensor_tensor(
                    stk2(lmh[:, 0], 3 * LS), stkb(t1, LS // 2),
                    stk2(th, 0), MIN)
                nc.vector.tensor_tensor(
                    stk2(lmh[:, 1], LS), stkb(t1, LS // 2),
                    stk2(lmh[:, 3], LS), MAX)

                # ---- horizontal pairs over (E,O) column planes, 2 slices
                # per instruction (FD=3072 @2x):
                #   melo[j]=max(loE,loO)  mxmd[j]=max(mdE,mdO)
                #   mnmd[j]=min(mdE,mdO)  mehi[j]=min(hiE,hiO)
                hp = pool.tile([RT, 4, 2, C, Wh], bf16, tag="hp", bufs=1)
                melo, mxmd, mnmd, mehi = hp[:, 0], hp[:, 1], hp[:, 2], hp[:, 3]
                nc.vector.tensor_tensor(
                    hp[:, 0:2], lmh[:, 0:2, :, :, 0], lmh[:, 0:2, :, :, 1],
                    MAX)
                nc.vector.tensor_tensor(
                    hp[:, 2:4], lmh[:, 1:3, :, :, 0], lmh[:, 1:3, :, :, 1],
                    MIN)

                # ---- ScalarE shifted copies into aligned scratch (the only
                # odd-offset reads; ScalarE is off the critical path).
                # sEO[0][k][j] = {lo,md,hi} E-plane[min(j+1, Wh-1)] (clamped)
                # sEO[1][k][j] = {lo,md,hi} O-plane[max(j-1, 0)]    (clamped)
                # The clamps make the full-width finals below reproduce the
                # horizontal reflect boundaries exactly (window {c,c',c}
                # median == clamp/max/min degenerate forms), so no separate
                # boundary-column pass is needed.
                sEO = pool.tile([RT, 2, 3, 2, C, Wh], bf16, tag="sEO",
                                bufs=2)
                # tiny edge clamps first (one op per column parity), then
                # the main shifts ordered lo -> md -> hi so each final op
                # below unblocks as soon as its own plane's scratch is ready
                nc.scalar.copy(sEO[:, 0, :, :, :, Wh - 1:Wh],
                               lmh[:, 0:3, :, :, 0, Wh - 1:Wh])
                nc.scalar.copy(sEO[:, 1, :, :, :, 0:1],
                               lmh[:, 0:3, :, :, 1, 0:1])
                for k in range(3):
                    nc.scalar.copy(sEO[:, 0, k, :, :, 0:Wh - 1],
                                   lmh[:, k, :, :, 0, 1:Wh])
                    nc.scalar.copy(sEO[:, 1, k, :, :, 1:Wh],
                                   lmh[:, k, :, :, 1, 0:Wh - 1])

                # ---- x/y/z tiles [2(col parity), 2(row parity), C, Wh]:
                # slice 0 = odd output cols 2j+1, slice 1 = even cols 2j.
                # Both column parities' finals run as ONE stacked op each:
                # the shared pair operand is broadcast over the parity dim
                # with a stride-0 AP; the single operand comes from sEO.
                x = pool.tile([RT, 2, 2, C, Wh], bf16, tag="x", bufs=1)
                y = pool.tile([RT, 2, 2, C, Wh], bf16, tag="y", bufs=1)
                tz = pool.tile([RT, 2, 2, 2, C, Wh], bf16, tag="tz", bufs=1)
                t, z = tz[:, 0], tz[:, 1]

                # odd cols 2j+1: pair (E[j],O[j]) + single E[j+1]
                # even cols 2j:  pair (E[j],O[j]) + single O[j-1]
                # The two MIN finals (t and z) stack into one double-FD op:
                # pair operand [mxmd ; mehi] (hp slices 1,3), single operand
                # [scr_md ; scr_hi] (sEO k slices 1,2).
                KS = 2 * C * Wh  # hp q-slice / sEO k-slice stride (1536)
                nc.vector.tensor_tensor(x[:], bcast2(melo), sEO[:, :, 0], MAX)
                h1 = hp[:, 1]
                s1 = sEO[:, :, 1]
                nc.vector.tensor_tensor(
                    tz[:],
                    AP(h1.tensor, h1.offset,
                       [list(h1.ap[0])] + [[2 * KS, 2], [0, 2]]
                       + [list(q) for q in h1.ap[1:]]),
                    AP(s1.tensor, s1.offset,
                       [list(s1.ap[0])] + [[KS, 2]]
                       + [list(q) for q in s1.ap[1:]]), MIN)
                nc.vector.tensor_tensor(y[:], bcast2(mnmd), t, MAX)

                # ---- final med3(x, y, z) (VectorE, FD=3072 @2x), then DMA
                # out (O col planes -> odd cols at +Wh, E -> even cols;
                # row parity rp selects dram row r0+2p+rp). The last step
                # runs med3+DMA per row parity to shorten the drain tail.
                f1 = pool.tile([RT, 2, 2, C, Wh], bf16, tag="f1", bufs=1)
                res = pool.tile([RT, 2, 2, C, Wh], bf16, tag="res")

                def med3_out(cs_, rs_):
                    xs, ys = x[:, cs_, rs_], y[:, cs_, rs_]
                    zs = tz[:, 1, cs_, rs_]
                    f1s, rr = f1[:, cs_, rs_], res[:, cs_, rs_]
                    nc.vector.tensor_tensor(f1s, xs, ys, MIN)
                    nc.vector.tensor_tensor(xs, xs, ys, MAX)
                    nc.vector.tensor_tensor(xs, xs, zs, MIN)
                    nc.vector.tensor_tensor(rr, f1s, xs, MAX)
                    for rp in range(2)[rs_]:
                        for cp, woff in (((0, Wh), (1, 0))[cs_]):
                            sp = res[:, cp, rp]
                            nc.sync.dma_start(
                                out=AP(out, (r0 + rp) * SR + b * SRB + woff,
                                       [[2 * SR, RT], [512, C], [1, Wh]]),
                                in_=AP(sp.tensor, sp.offset,
                                       [list(sp.ap[0])] + [[Wh, C],
                                                           [1, Wh]]))

                full = slice(None)
                if it == len(steps) - 1:
                    # drain-tail: finest split on the very last chunk so the
                    # final output transfer overlaps the last med3 compute
                    med3_out(full, slice(0, 1))
                    for cp in range(2):
                        med3_out(slice(cp, cp + 1), slice(1, 2))
                else:
                    med3_out(full, full)

    _legalize_waits(nc, mybir)
    return nc


def _stage_input(img_k: np.ndarray) -> np.ndarray:
    """[BPC, C, H, W] f32 -> [H+2, BPC, C, 2, W/2] bf16: batches merged
    into each row, columns deinterleaved into even/odd planes, vertical
    reflect rows pre-staged."""
    import ml_dtypes
    t = img_k.astype(ml_dtypes.bfloat16)
    # [H, BPC, C, 2(eo), Wh]
    v = t.reshape(BPC, C, H, Wh, 2).transpose(2, 0, 1, 4, 3)
    p = np.empty((HP, BPC, C, 2, Wh), dtype=ml_dtypes.bfloat16)
    p[1:H + 1] = v
    p[0] = v[1]          # reflect: row -1 = row 1
    p[H + 1] = v[H - 2]  # reflect: row H = row H-2
    return np.ascontiguousarray(p)


def _unstage_output(res_k: np.ndarray) -> np.ndarray:
    """[H, BPC, C, 2, W/2] bf16 -> [BPC, C, H, W] f32 (reinterleave)."""
    r = res_k.transpose(1, 2, 0, 4, 3)  # [BPC, C, H, Wh, 2]
    return r.reshape(BPC, C, H, W).astype(np.float32)


def kernel(image: np.ndarray) -> np.ndarray:
    from concourse.bass_utils import run_bass_kernel_spmd

    image = np.asarray(image, dtype=np.float32)
    if "nc" not in _COMPILED:
        _COMPILED["nc"] = _build_nc()
    nc = _COMPILED["nc"]

    in_maps = [{"image": _stage_input(image[k * BPC:(k + 1) * BPC])}
               for k in range(NCORES)]
    for attempt in range(3):
        try:
            res = run_bass_kernel_spmd(nc, in_maps,
                                       core_ids=list(range(NCORES)))
            break
        except Exception:
            # transient accelerator errors (e.g. NRT_EXEC_UNIT_UNRECOVERABLE)
            # have been observed to clear on retry
            if attempt == 2:
                raise
            import time
            time.sleep(10)
    return np.concatenate(
        [_unstage_output(res.results[k]["out"]) for k in range(NCORES)],
        axis=0)


# revision 65
# speedup vs baseline: 1.0054x; 1.0054x over previous
"""3x3 median filter (reflect padding) on Trainium2, data-parallel over batch.

Input:  image [16, 3, 512, 512] f32
Output: same shape; out[b,c,y,x] = median of the 3x3 window around (y,x),
        reflect padding.

Sharding: batch dim split across 8 NeuronCores (2 images per core), SPMD.

Compute runs in bf16 (rel err ~4e-3, within tolerance). The key TRN2 fact:
VectorE TENSOR_TENSOR runs at 2 elem/cycle (2x_1P mode) only for 16-bit
dtypes with innermost stride +-1 AND 4-byte-aligned streams; any stride-2
or odd-element-shifted operand falls back to 1 elem/cycle. The horizontal
median stage needs column-neighbor access, so:

Host prep: per-core input is staged to [H+2, BPC, C, 2, W/2] bf16 (both
images merged into each padded row so every DMA access pattern stays
3-dim) with even/odd columns DEINTERLEAVED (E plane = cols 0,2,...,
O plane = cols 1,3,...) and the two vertical reflect rows pre-staged.
Every horizontal op then reads aligned plane pairs, and the only shifted
(odd-offset) reads are done by the otherwise-idle ScalarE as clamped
copies into aligned scratch; every VectorE op runs at 2x.

Per-core algorithm (separable exact median with stride-2 vertical pair
sharing, per output pixel amortized: 5 vertical + 2 pair + 4 final +
4 med3 = 15 VectorE min/max elem-ops). Each SBUF partition owns a ROW
PAIR (padded rows r0+2p+1, r0+2p+2) shared by the two output rows
r0+2p (third row below) and r0+2p+1 (third row above), so the t1/t2
sort ops run once per pair and each padded row is DMA'd twice, not 3x.
4 steps of (one image, 256-row band); row parity rp lives on the free
axis; all TENSOR_TENSOR at 2x mode:
  1. Load pairT [128,2,C,2,Wh] + both third rows th [128,2,C,2,Wh]
     (2 DMAs; partition stride 2 rows, th's rp-dim stride 3 rows).
  2. t1/t2 = min/max of the pair (2 TT, FD=1536), then both parities'
     third-row merges stacked per op via stride-0 broadcast of t1/t2
     over rp: m, hi, lo, md (4 TT, FD=3072).
  3. Horizontal pairs on E/O column planes: melo,mxmd = max over
     (lo,md); mnmd,mehi = min over (md,hi) (2 stacked TT, FD=3072)
  4. ScalarE: sEO[0][k] = E-plane of {lo,md,hi} shifted left, clamped at
     the edge (for odd output cols); sEO[1][k] = O-plane shifted right,
     clamped (for even cols). The clamps reproduce the horizontal
     reflect boundary columns exactly (median of a {c,c',c} window
     degenerates to the pair reduction), so there is no boundary pass.
     Copies run per plane, ordered lo->md->hi, in the VectorE shadow.
  5. Finals, both column parities fused per op by broadcasting the
     shared pair operand over the parity dim with a stride-0 AP (4 TT,
     FD=3072):
       odd  col 2j+1: X=max(melo[j],loE[j+1]) Z=min(mehi[j],hiE[j+1])
                      Y=max(mnmd[j],min(mxmd[j],mdE[j+1]))
       even col 2j:   same with the single taken from O[j-1]
  6. median = med3(X, Y, Z) (4 TT, FD=3072); DMA out E/O planes per rp
     (the last step runs med3+DMA per rp to shorten the drain tail).

Measured on HW: 234.8 us (f32 1x baseline) -> 119.4 us. VectorE is the
bottleneck at ~97% busy; its TENSOR_TENSOR floor for this op count is
~106.4 us, plus ~7 us fixed engine preamble, ~5 us initial DMA fill and
~5.5 us drain tail.
"""

import sys

sys.path.insert(0, "/opt/trn_rl_repo")

import numpy as np

_COMPILED = {}

B, C, H, W = 16, 3, 512, 512
NCORES = 8
BPC = B // NCORES  # batches per core (stacked on the free axis)
RT = 128           # output rows per step
NRT = H // RT      # steps (each covers all BPC batches)
HP = H + 2         # padded rows on device
Wh = W // 2        # half width (E/O plane width)
SR = BPC * C * W   # padded-row stride (elements) in device layout
                   # [HP, BPC, C, 2, Wh] -- both batches live in one row


def _legalize_waits(nc, mybir):
    """Hoist excess sync-waits into a preceding same-engine EventSemaphore.
    The TRN2 ISA allows 1 sync-wait on compute instructions (2 on DMACopy;
    EventSemaphore allows several) but Tile's scheduler can emit more; a
    wait-only instruction earlier in the same engine's program order is
    semantically identical."""
    limits = {"InstEventSemaphore": 2}
    n_hoisted = 0
    for f in nc.m.functions:
        for bb in f.blocks:
            il = bb.instructions
            idx = 0
            while idx < len(il):
                i = il[idx]
                si = i.sync_info
                lim = limits.get(type(i).__name__, 1)
                if si is not None and si.on_wait and len(si.on_wait) > lim:
                    waits = list(si.on_wait)
                    keep, excess = waits[:lim], waits[lim:]
                    hoists = []
                    for j in range(0, len(excess), 2):
                        h = mybir.InstEventSemaphore(
                            name=f"hoistw_{n_hoisted}", ins=[], outs=[])
                        n_hoisted += 1
                        h.engine = i.engine
                        h.sync_info = mybir.SyncInfo(
                            on_wait=excess[j:j + 2], on_update=[])
                        hoists.append(h)
                    i.sync_info = mybir.SyncInfo(
                        on_wait=keep, on_update=si.on_update)
                    for k, h in enumerate(hoists):
                        il.insert(idx + k, h)
                    idx += len(hoists)
                idx += 1
    return n_hoisted


def _build_nc():
    from concourse import bass
    import concourse.mybir as mybir
    from concourse.tile import TileContext

    bf16 = mybir.dt.bfloat16
    MIN = mybir.AluOpType.min
    MAX = mybir.AluOpType.max
    AP = bass.AP

    nc = bass.Bass()
    img = nc.dram_tensor("image", [HP, BPC, C, 2, Wh], bf16,
                         kind="ExternalInput")
    out = nc.dram_tensor("out", [H, BPC, C, 2, Wh], bf16,
                         kind="ExternalOutput")

    SRB = C * W  # per-batch chunk of a padded row (1536)
    RB = 2 * RT  # output rows per step (each partition owns a ROW PAIR)
    steps = [(b, band) for b in range(BPC) for band in range(H // RB)]
    with TileContext(nc) as tc:
        with tc.tile_pool(name="p", bufs=2) as pool:
            for it, (b, band) in enumerate(steps):
                r0 = band * RB
                # ---- stride-2 vertical pair sharing: partition p owns the
                # row PAIR (padded rows r0+2p+1, r0+2p+2), which is shared
                # by output rows r0+2p (third = padded r0+2p) and r0+2p+1
                # (third = padded r0+2p+3). Each padded row is loaded twice
                # (not 3x) and the t1/t2 sort ops run once per pair.
                pairT = pool.tile([RT, 2, C, 2, Wh], bf16, tag="pair",
                                  bufs=2)
                th = pool.tile([RT, 2, C, 2, Wh], bf16, tag="th", bufs=2)
                nc.sync.dma_start(out=pairT[:], in_=AP(
                    img, (r0 + 1) * SR + b * SRB,
                    [[2 * SR, RT], [SR, 2], [1, SRB]]))
                # both third rows in one DMA: rp=0 -> row r0+2p (below),
                # rp=1 -> row r0+2p+3 (above); rp-dim stride = 3 rows
                nc.sync.dma_start(out=th[:], in_=AP(
                    img, r0 * SR + b * SRB,
                    [[2 * SR, RT], [3 * SR, 2], [1, SRB]]))

                def bcast2(h):
                    # insert a stride-0 broadcast dim after the partition dim
                    return AP(h.tensor, h.offset,
                              [list(h.ap[0])] + [[0, 2]] +
                              [list(q) for q in h.ap[1:]])

                # ---- vertical sort3 (t1/t2 once per pair, FD=1536 @2x;
                # both parities' third-row merges stacked per op by
                # broadcasting t1/t2 over rp, FD=3072 @2x)
                # t1/t2 in one tile so [lo ; m] can stack: lo = min(t1, th)
                # (== min(t1, m) since t1 <= t2) and m = min(t2, th) are
                # independent MINs sharing th -> one double-FD op. m lives
                # as lmh slice 3 (scratch, only md reads it).
                t12 = pool.tile([RT, 2, C, 2, Wh], bf16, tag="t12", bufs=1)
                t1, t2 = t12[:, 0], t12[:, 1]
                lmh = pool.tile([RT, 4, 2, C, 2, Wh], bf16, tag="lmh",
                                bufs=1)
                nc.vector.tensor_tensor(t1, pairT[:, 0], pairT[:, 1], MIN)
                nc.vector.tensor_tensor(t2, pairT[:, 0], pairT[:, 1], MAX)
                LS = 2 * C * 2 * Wh  # lmh slice stride (3072)
                l0 = lmh[:, 0]
                nc.vector.tensor_tensor(
                    AP(l0.tensor, l0.offset,
                       [list(l0.ap[0])] + [[3 * LS, 2]]
                       + [list(q) for q in l0.ap[1:]]),
                    AP(t12.tensor, t12[:, 0].offset,
                       [list(t12[:, 0].ap[0])] + [[LS // 2, 2], [0, 2]]
                       + [list(q) for q in t12[:, 0].ap[1:]]),
                    AP(th.tensor, th[:].offset,
                       [list(th[:].ap[0])] + [[0, 2]]
                       + [list(q) for q in th[:].ap[1:]]), MIN)
                nc.vector.tensor_tensor(lmh[:, 2], bcast2(t2), th[:], MAX)
                nc.vector.tensor_tensor(lmh[:, 1], bcast2(t1), lmh[:, 3],
                                        MAX)

                # ---- horizontal pairs over (E,O) column planes, 2 slices
                # per instruction (FD=3072 @2x):
                #   melo[j]=max(loE,loO)  mxmd[j]=max(mdE,mdO)
                #   mnmd[j]=min(mdE,mdO)  mehi[j]=min(hiE,hiO)
                hp = pool.tile([RT, 4, 2, C, Wh], bf16, tag="hp", bufs=1)
                melo, mxmd, mnmd, mehi = hp[:, 0], hp[:, 1], hp[:, 2], hp[:, 3]
                nc.vector.tensor_tensor(
                    hp[:, 0:2], lmh[:, 0:2, :, :, 0], lmh[:, 0:2, :, :, 1],
                    MAX)
                nc.vector.tensor_tensor(
                    hp[:, 2:4], lmh[:, 1:3, :, :, 0], lmh[:, 1:3, :, :, 1],
                    MIN)

                # ---- ScalarE shifted copies into aligned scratch (the only
                # odd-offset reads; ScalarE is off the critical path).
                # sEO[0][k][j] = {lo,md,hi} E-plane[min(j+1, Wh-1)] (clamped)
                # sEO[1][k][j] = {lo,md,hi} O-plane[max(j-1, 0)]    (clamped)
                # The clamps make the full-width finals below reproduce the
                # horizontal reflect boundaries exactly (window {c,c',c}
                # median == clamp/max/min degenerate forms), so no separate
                # boundary-column pass is needed.
                sEO = pool.tile([RT, 2, 3, 2, C, Wh], bf16, tag="sEO",
                                bufs=2)
                # tiny edge clamps first (one op per column parity), then
                # the main shifts ordered lo -> md -> hi so each final op
                # below unblocks as soon as its own plane's scratch is ready
                nc.scalar.copy(sEO[:, 0, :, :, :, Wh - 1:Wh],
                               lmh[:, 0:3, :, :, 0, Wh - 1:Wh])
                nc.scalar.copy(sEO[:, 1, :, :, :, 0:1],
                               lmh[:, 0:3, :, :, 1, 0:1])
                for k in range(3):
                    nc.scalar.copy(sEO[:, 0, k, :, :, 0:Wh - 1],
                                   lmh[:, k, :, :, 0, 1:Wh])
                    nc.scalar.copy(sEO[:, 1, k, :, :, 1:Wh],
                                   lmh[:, k, :, :, 1, 0:Wh - 1])

                # ---- x/y/z tiles [2(col parity), 2(row parity), C, Wh]:
                # slice 0 = odd output cols 2j+1, slice 1 = even cols 2j.
                # Both column parities' finals run as ONE stacked op each:
                # the shared pair operand is broadcast over the parity dim
                # with a stride-0 AP; the single operand comes from sEO.
                x = pool.tile([RT, 2, 2, C, Wh], bf16, tag="x", bufs=1)
                y = pool.tile([RT, 2, 2, C, Wh], bf16, tag="y", bufs=1)
                tz = pool.tile([RT, 2, 2, 2, C, Wh], bf16, tag="tz", bufs=1)
                t, z = tz[:, 0], tz[:, 1]

                # odd cols 2j+1: pair (E[j],O[j]) + single E[j+1]
                # even cols 2j:  pair (E[j],O[j]) + single O[j-1]
                # The two MIN finals (t and z) stack into one double-FD op:
                # pair operand [mxmd ; mehi] (hp slices 1,3), single operand
                # [scr_md ; scr_hi] (sEO k slices 1,2).
                KS = 2 * C * Wh  # hp q-slice / sEO k-slice stride (1536)
                nc.vector.tensor_tensor(x[:], bcast2(melo), sEO[:, :, 0], MAX)
                h1 = hp[:, 1]
                s1 = sEO[:, :, 1]
                nc.vector.tensor_tensor(
                    tz[:],
                    AP(h1.tensor, h1.offset,
                       [list(h1.ap[0])] + [[2 * KS, 2], [0, 2]]
                       + [list(q) for q in h1.ap[1:]]),
                    AP(s1.tensor, s1.offset,
                       [list(s1.ap[0])] + [[KS, 2]]
                       + [list(q) for q in s1.ap[1:]]), MIN)
                nc.vector.tensor_tensor(y[:], bcast2(mnmd), t, MAX)

                # ---- final med3(x, y, z) (VectorE, FD=3072 @2x), then DMA
                # out (O col planes -> odd cols at +Wh, E -> even cols;
                # row parity rp selects dram row r0+2p+rp). The last step
                # runs med3+DMA per row parity to shorten the drain tail.
                f1 = pool.tile([RT, 2, 2, C, Wh], bf16, tag="f1", bufs=1)
                res = pool.tile([RT, 2, 2, C, Wh], bf16, tag="res")

                def med3_out(cs_, rs_):
                    xs, ys = x[:, cs_, rs_], y[:, cs_, rs_]
                    zs = tz[:, 1, cs_, rs_]
                    f1s, rr = f1[:, cs_, rs_], res[:, cs_, rs_]
                    nc.vector.tensor_tensor(f1s, xs, ys, MIN)
                    nc.vector.tensor_tensor(xs, xs, ys, MAX)
                    nc.vector.tensor_tensor(xs, xs, zs, MIN)
                    nc.vector.tensor_tensor(rr, f1s, xs, MAX)
                    for rp in range(2)[rs_]:
                        for cp, woff in (((0, Wh), (1, 0))[cs_]):
                            sp = res[:, cp, rp]
                            nc.sync.dma_start(
                                out=AP(out, (r0 + rp) * SR + b * SRB + woff,
                                       [[2 * SR, RT], [512, C], [1, Wh]]),
                                in_=AP(sp.tensor, sp.offset,
                                       [list(sp.ap[0])] + [[Wh, C],
                                                           [1, Wh]]))

                full = slice(None)
                if it == len(steps) - 1:
                    # drain-tail: finest split on the very last chunk so the
                    # final output transfer overlaps the last med3 compute
                    med3_out(full, slice(0, 1))
                    for cp in range(2):
                        med3_out(slice(cp, cp + 1), slice(1, 2))
                else:
                    med3_out(full, full)

    _legalize_waits(nc, mybir)
    return nc


def _stage_input(img_k: np.ndarray) -> np.ndarray:
    """[BPC, C, H, W] f32 -> [H+2, BPC, C, 2, W/2] bf16: batches merged
    into each row, columns deinterleaved into even/odd planes, vertical
    reflect rows pre-staged."""
    import ml_dtypes
    t = img_k.astype(ml_dtypes.bfloat16)
    # [H, BPC, C, 2(eo), Wh]
    v = t.reshape(BPC, C, H, Wh, 2).transpose(2, 0, 1, 4, 3)
    p = np.empty((HP, BPC, C, 2, Wh), dtype=ml_dtypes.bfloat16)
    p[1:H + 1] = v
    p[0] = v[1]          # reflect: row -1 = row 1
    p[H + 1] = v[H - 2]  # reflect: row H = row H-2
    return np.ascontiguousarray(p)


def _unstage_output(res_k: np.ndarray) -> np.ndarray:
    """[H, BPC, C, 2, W/2] bf16 -> [BPC, C, H, W] f32 (reinterleave)."""
    r = res_k.transpose(1, 2, 0, 4, 3)  # [BPC, C, H, Wh, 2]
    return r.reshape(BPC, C, H, W).astype(np.float32)


def kernel(image: np.ndarray) -> np.ndarray:
    from concourse.bass_utils import run_bass_kernel_spmd

    image = np.asarray(image, dtype=np.float32)
    if "nc" not in _COMPILED:
        _COMPILED["nc"] = _build_nc()
    nc = _COMPILED["nc"]

    in_maps = [{"image": _stage_input(image[k * BPC:(k + 1) * BPC])}
               for k in range(NCORES)]
    for attempt in range(3):
        try:
            res = run_bass_kernel_spmd(nc, in_maps,
                                       core_ids=list(range(NCORES)))
            break
        except Exception:
            # transient accelerator errors (e.g. NRT_EXEC_UNIT_UNRECOVERABLE)
            # have been observed to clear on retry
            if attempt == 2:
                raise
            import time
            time.sleep(10)
    return np.concatenate(
        [_unstage_output(res.results[k]["out"]) for k in range(NCORES)],
        axis=0)


# revision 66
# speedup vs baseline: 1.0061x; 1.0007x over previous
"""3x3 median filter (reflect padding) on Trainium2, data-parallel over batch.

Input:  image [16, 3, 512, 512] f32
Output: same shape; out[b,c,y,x] = median of the 3x3 window around (y,x),
        reflect padding.

Sharding: batch dim split across 8 NeuronCores (2 images per core), SPMD.

Compute runs in bf16 (rel err ~4e-3, within tolerance). The key TRN2 fact:
VectorE TENSOR_TENSOR runs at 2 elem/cycle (2x_1P mode) only for 16-bit
dtypes with innermost stride +-1 AND 4-byte-aligned streams; any stride-2
or odd-element-shifted operand falls back to 1 elem/cycle. The horizontal
median stage needs column-neighbor access, so:

Host prep: per-core input is staged to [H+2, BPC, C, 2, W/2] bf16 (both
images merged into each padded row so every DMA access pattern stays
3-dim) with even/odd columns DEINTERLEAVED (E plane = cols 0,2,...,
O plane = cols 1,3,...) and the two vertical reflect rows pre-staged.
Every horizontal op then reads aligned plane pairs, and the only shifted
(odd-offset) reads are done by the otherwise-idle ScalarE as clamped
copies into aligned scratch; every VectorE op runs at 2x.

Per-core algorithm (separable exact median with stride-2 vertical pair
sharing, per output pixel amortized: 5 vertical + 2 pair + 4 final +
4 med3 = 15 VectorE min/max elem-ops). Each SBUF partition owns a ROW
PAIR (padded rows r0+2p+1, r0+2p+2) shared by the two output rows
r0+2p (third row below) and r0+2p+1 (third row above), so the t1/t2
sort ops run once per pair and each padded row is DMA'd twice, not 3x.
4 steps of (one image, 256-row band); row parity rp lives on the free
axis; all TENSOR_TENSOR at 2x mode:
  1. Load pairT [128,2,C,2,Wh] + both third rows th [128,2,C,2,Wh]
     (2 DMAs; partition stride 2 rows, th's rp-dim stride 3 rows).
  2. t1/t2 = min/max of the pair (2 TT, FD=1536), then both parities'
     third-row merges stacked per op via stride-0 broadcast of t1/t2
     over rp: m, hi, lo, md (4 TT, FD=3072).
  3. Horizontal pairs on E/O column planes: melo,mxmd = max over
     (lo,md); mnmd,mehi = min over (md,hi) (2 stacked TT, FD=3072)
  4. ScalarE: sEO[0][k] = E-plane of {lo,md,hi} shifted left, clamped at
     the edge (for odd output cols); sEO[1][k] = O-plane shifted right,
     clamped (for even cols). The clamps reproduce the horizontal
     reflect boundary columns exactly (median of a {c,c',c} window
     degenerates to the pair reduction), so there is no boundary pass.
     Copies run per plane, ordered lo->md->hi, in the VectorE shadow.
  5. Finals, both column parities fused per op by broadcasting the
     shared pair operand over the parity dim with a stride-0 AP (4 TT,
     FD=3072):
       odd  col 2j+1: X=max(melo[j],loE[j+1]) Z=min(mehi[j],hiE[j+1])
                      Y=max(mnmd[j],min(mxmd[j],mdE[j+1]))
       even col 2j:   same with the single taken from O[j-1]
  6. median = med3(X, Y, Z) (4 TT, FD=3072); DMA out E/O planes per rp
     (the last step runs med3+DMA per rp to shorten the drain tail).

Measured on HW: 234.8 us (f32 1x baseline) -> 119.4 us. VectorE is the
bottleneck at ~97% busy; its TENSOR_TENSOR floor for this op count is
~106.4 us, plus ~7 us fixed engine preamble, ~5 us initial DMA fill and
~5.5 us drain tail.
"""

import sys

sys.path.insert(0, "/opt/trn_rl_repo")

import numpy as np

_COMPILED = {}

B, C, H, W = 16, 3, 512, 512
NCORES = 8
BPC = B // NCORES  # batches per core (stacked on the free axis)
RT = 128           # output rows per step
NRT = H // RT      # steps (each covers all BPC batches)
HP = H + 2         # padded rows on device
Wh = W // 2        # half width (E/O plane width)
SR = BPC * C * W   # padded-row stride (elements) in device layout
                   # [HP, BPC, C, 2, Wh] -- both batches live in one row


def _legalize_waits(nc, mybir):
    """Hoist excess sync-waits into a preceding same-engine EventSemaphore.
    The TRN2 ISA allows 1 sync-wait on compute instructions (2 on DMACopy;
    EventSemaphore allows several) but Tile's scheduler can emit more; a
    wait-only instruction earlier in the same engine's program order is
    semantically identical."""
    limits = {"InstEventSemaphore": 2}
    n_hoisted = 0
    for f in nc.m.functions:
        for bb in f.blocks:
            il = bb.instructions
            idx = 0
            while idx < len(il):
                i = il[idx]
                si = i.sync_info
                lim = limits.get(type(i).__name__, 1)
                if si is not None and si.on_wait and len(si.on_wait) > lim:
                    waits = list(si.on_wait)
                    keep, excess = waits[:lim], waits[lim:]
                    hoists = []
                    for j in range(0, len(excess), 2):
                        h = mybir.InstEventSemaphore(
                            name=f"hoistw_{n_hoisted}", ins=[], outs=[])
                        n_hoisted += 1
                        h.engine = i.engine
                        h.sync_info = mybir.SyncInfo(
                            on_wait=excess[j:j + 2], on_update=[])
                        hoists.append(h)
                    i.sync_info = mybir.SyncInfo(
                        on_wait=keep, on_update=si.on_update)
                    for k, h in enumerate(hoists):
                        il.insert(idx + k, h)
                    idx += len(hoists)
                idx += 1
    return n_hoisted


def _build_nc():
    from concourse import bass
    import concourse.mybir as mybir
    from concourse.tile import TileContext

    bf16 = mybir.dt.bfloat16
    MIN = mybir.AluOpType.min
    MAX = mybir.AluOpType.max
    AP = bass.AP

    nc = bass.Bass()
    img = nc.dram_tensor("image", [HP, BPC, C, 2, Wh], bf16,
                         kind="ExternalInput")
    out = nc.dram_tensor("out", [H, BPC, C, 2, Wh], bf16,
                         kind="ExternalOutput")

    SRB = C * W  # per-batch chunk of a padded row (1536)
    RB = 2 * RT  # output rows per step (each partition owns a ROW PAIR)
    steps = [(b, band) for b in range(BPC) for band in range(H // RB)]
    with TileContext(nc) as tc:
        with tc.tile_pool(name="p", bufs=2) as pool:
            for it, (b, band) in enumerate(steps):
                r0 = band * RB
                # ---- stride-2 vertical pair sharing: partition p owns the
                # row PAIR (padded rows r0+2p+1, r0+2p+2), which is shared
                # by output rows r0+2p (third = padded r0+2p) and r0+2p+1
                # (third = padded r0+2p+3). Each padded row is loaded twice
                # (not 3x) and the t1/t2 sort ops run once per pair.
                pairT = pool.tile([RT, 2, C, 2, Wh], bf16, tag="pair",
                                  bufs=2)
                th = pool.tile([RT, 2, C, 2, Wh], bf16, tag="th", bufs=2)
                nc.sync.dma_start(out=pairT[:], in_=AP(
                    img, (r0 + 1) * SR + b * SRB,
                    [[2 * SR, RT], [SR, 2], [1, SRB]]))
                # both third rows in one DMA: rp=0 -> row r0+2p (below),
                # rp=1 -> row r0+2p+3 (above); rp-dim stride = 3 rows
                nc.sync.dma_start(out=th[:], in_=AP(
                    img, r0 * SR + b * SRB,
                    [[2 * SR, RT], [3 * SR, 2], [1, SRB]]))

                def bcast2(h):
                    # insert a stride-0 broadcast dim after the partition dim
                    return AP(h.tensor, h.offset,
                              [list(h.ap[0])] + [[0, 2]] +
                              [list(q) for q in h.ap[1:]])

                # ---- vertical sort3 (t1/t2 once per pair, FD=1536 @2x;
                # both parities' third-row merges stacked per op by
                # broadcasting t1/t2 over rp, FD=3072 @2x)
                # t1/t2 in one tile so [lo ; m] can stack: lo = min(t1, th)
                # (== min(t1, m) since t1 <= t2) and m = min(t2, th) are
                # independent MINs sharing th -> one double-FD op. m lives
                # as lmh slice 3 (scratch, only md reads it).
                t12 = pool.tile([RT, 2, C, 2, Wh], bf16, tag="t12", bufs=1)
                t1, t2 = t12[:, 0], t12[:, 1]
                lmh = pool.tile([RT, 4, 2, C, 2, Wh], bf16, tag="lmh",
                                bufs=1)
                nc.vector.tensor_tensor(t1, pairT[:, 0], pairT[:, 1], MIN)
                nc.vector.tensor_tensor(t2, pairT[:, 0], pairT[:, 1], MAX)
                LS = 2 * C * 2 * Wh  # lmh slice stride (3072)
                l0 = lmh[:, 0]
                nc.vector.tensor_tensor(
                    AP(l0.tensor, l0.offset,
                       [list(l0.ap[0])] + [[3 * LS, 2]]
                       + [list(q) for q in l0.ap[1:]]),
                    AP(t12.tensor, t12[:, 0].offset,
                       [list(t12[:, 0].ap[0])] + [[LS // 2, 2], [0, 2]]
                       + [list(q) for q in t12[:, 0].ap[1:]]),
                    AP(th.tensor, th[:].offset,
                       [list(th[:].ap[0])] + [[0, 2]]
                       + [list(q) for q in th[:].ap[1:]]), MIN)
                nc.vector.tensor_tensor(lmh[:, 2], bcast2(t2), th[:], MAX)
                nc.vector.tensor_tensor(lmh[:, 1], bcast2(t1), lmh[:, 3],
                                        MAX)

                # ---- horizontal pairs over (E,O) column planes, 2 slices
                # per instruction (FD=3072 @2x):
                #   melo[j]=max(loE,loO)  mxmd[j]=max(mdE,mdO)
                #   mnmd[j]=min(mdE,mdO)  mehi[j]=min(hiE,hiO)
                hp = pool.tile([RT, 4, 2, C, Wh], bf16, tag="hp", bufs=1)
                melo, mxmd, mnmd, mehi = hp[:, 0], hp[:, 1], hp[:, 2], hp[:, 3]
                nc.vector.tensor_tensor(
                    hp[:, 0:2], lmh[:, 0:2, :, :, 0], lmh[:, 0:2, :, :, 1],
                    MAX)
                nc.vector.tensor_tensor(
                    hp[:, 2:4], lmh[:, 1:3, :, :, 0], lmh[:, 1:3, :, :, 1],
                    MIN)

                # ---- ScalarE shifted copies into aligned scratch (the only
                # odd-offset reads; ScalarE is off the critical path).
                # sEO[0][k][j] = {lo,md,hi} E-plane[min(j+1, Wh-1)] (clamped)
                # sEO[1][k][j] = {lo,md,hi} O-plane[max(j-1, 0)]    (clamped)
                # The clamps make the full-width finals below reproduce the
                # horizontal reflect boundaries exactly (window {c,c',c}
                # median == clamp/max/min degenerate forms), so no separate
                # boundary-column pass is needed.
                sEO = pool.tile([RT, 2, 3, 2, C, Wh], bf16, tag="sEO",
                                bufs=2)
                # tiny edge clamps first (one op per column parity), then
                # the main shifts ordered lo -> md -> hi so each final op
                # below unblocks as soon as its own plane's scratch is ready
                nc.scalar.copy(sEO[:, 0, :, :, :, Wh - 1:Wh],
                               lmh[:, 0:3, :, :, 0, Wh - 1:Wh])
                nc.scalar.copy(sEO[:, 1, :, :, :, 0:1],
                               lmh[:, 0:3, :, :, 1, 0:1])
                for k in range(3):
                    nc.scalar.copy(sEO[:, 0, k, :, :, 0:Wh - 1],
                                   lmh[:, k, :, :, 0, 1:Wh])
                    nc.scalar.copy(sEO[:, 1, k, :, :, 1:Wh],
                                   lmh[:, k, :, :, 1, 0:Wh - 1])

                # ---- x/y/z tiles [2(col parity), 2(row parity), C, Wh]:
                # slice 0 = odd output cols 2j+1, slice 1 = even cols 2j.
                # Both column parities' finals run as ONE stacked op each:
                # the shared pair operand is broadcast over the parity dim
                # with a stride-0 AP; the single operand comes from sEO.
                x = pool.tile([RT, 2, 2, C, Wh], bf16, tag="x", bufs=1)
                y = pool.tile([RT, 2, 2, C, Wh], bf16, tag="y", bufs=1)
                tz = pool.tile([RT, 2, 2, 2, C, Wh], bf16, tag="tz", bufs=1)
                t, z = tz[:, 0], tz[:, 1]

                # odd cols 2j+1: pair (E[j],O[j]) + single E[j+1]
                # even cols 2j:  pair (E[j],O[j]) + single O[j-1]
                # The two MIN finals (t and z) stack into one double-FD op:
                # pair operand [mxmd ; mehi] (hp slices 1,3), single operand
                # [scr_md ; scr_hi] (sEO k slices 1,2).
                KS = 2 * C * Wh  # hp q-slice / sEO k-slice stride (1536)
                nc.vector.tensor_tensor(x[:], bcast2(melo), sEO[:, :, 0], MAX)
                h1 = hp[:, 1]
                s1 = sEO[:, :, 1]
                nc.vector.tensor_tensor(
                    tz[:],
                    AP(h1.tensor, h1.offset,
                       [list(h1.ap[0])] + [[2 * KS, 2], [0, 2]]
                       + [list(q) for q in h1.ap[1:]]),
                    AP(s1.tensor, s1.offset,
                       [list(s1.ap[0])] + [[KS, 2]]
                       + [list(q) for q in s1.ap[1:]]), MIN)
                nc.vector.tensor_tensor(y[:], bcast2(mnmd), t, MAX)

                # ---- final med3(x, y, z) (VectorE, FD=3072 @2x), then DMA
                # out (O col planes -> odd cols at +Wh, E -> even cols;
                # row parity rp selects dram row r0+2p+rp). The last step
                # runs med3+DMA per row parity to shorten the drain tail.
                f1 = pool.tile([RT, 2, 2, C, Wh], bf16, tag="f1", bufs=1)
                res = pool.tile([RT, 2, 2, C, Wh], bf16, tag="res", bufs=1)

                def med3_out(cs_, rs_):
                    xs, ys = x[:, cs_, rs_], y[:, cs_, rs_]
                    zs = tz[:, 1, cs_, rs_]
                    f1s, rr = f1[:, cs_, rs_], res[:, cs_, rs_]
                    nc.vector.tensor_tensor(f1s, xs, ys, MIN)
                    nc.vector.tensor_tensor(xs, xs, ys, MAX)
                    nc.vector.tensor_tensor(xs, xs, zs, MIN)
                    nc.vector.tensor_tensor(rr, f1s, xs, MAX)
                    for rp in range(2)[rs_]:
                        for cp, woff in (((0, Wh), (1, 0))[cs_]):
                            sp = res[:, cp, rp]
                            nc.sync.dma_start(
                                out=AP(out, (r0 + rp) * SR + b * SRB + woff,
                                       [[2 * SR, RT], [512, C], [1, Wh]]),
                                in_=AP(sp.tensor, sp.offset,
                                       [list(sp.ap[0])] + [[Wh, C],
                                                           [1, Wh]]))

                full = slice(None)
                if it == len(steps) - 1:
                    # drain-tail: finest split on the very last chunk so the
                    # final output transfer overlaps the last med3 compute
                    med3_out(full, slice(0, 1))
                    for cp in range(2):
                        med3_out(slice(cp, cp + 1), slice(1, 2))
                else:
                    med3_out(full, full)

    _legalize_waits(nc, mybir)
    return nc


def _stage_input(img_k: np.ndarray) -> np.ndarray:
    """[BPC, C, H, W] f32 -> [H+2, BPC, C, 2, W/2] bf16: batches merged
    into each row, columns deinterleaved into even/odd planes, vertical
    reflect rows pre-staged."""
    import ml_dtypes
    t = img_k.astype(ml_dtypes.bfloat16)
    # [H, BPC, C, 2(eo), Wh]
    v = t.reshape(BPC, C, H, Wh, 2).transpose(2, 0, 1, 4, 3)
    p = np.empty((HP, BPC, C, 2, Wh), dtype=ml_dtypes.bfloat16)
    p[1:H + 1] = v
    p[0] = v[1]          # reflect: row -1 = row 1
    p[H + 1] = v[H - 2]  # reflect: row H = row H-2
    return np.ascontiguousarray(p)


def _unstage_output(res_k: np.ndarray) -> np.ndarray:
    """[H, BPC, C, 2, W/2] bf16 -> [BPC, C, H, W] f32 (reinterleave)."""
    r = res_k.transpose(1, 2, 0, 4, 3)  # [BPC, C, H, Wh, 2]
    return r.reshape(BPC, C, H, W).astype(np.float32)


def kernel(image: np.ndarray) -> np.ndarray:
    from concourse.bass_utils import run_bass_kernel_spmd

    image = np.asarray(image, dtype=np.float32)
    if "nc" not in _COMPILED:
        _COMPILED["nc"] = _build_nc()
    nc = _COMPILED["nc"]

    in_maps = [{"image": _stage_input(image[k * BPC:(k + 1) * BPC])}
               for k in range(NCORES)]
    for attempt in range(3):
        try:
            res = run_bass_kernel_spmd(nc, in_maps,
                                       core_ids=list(range(NCORES)))
            break
        except Exception:
            # transient accelerator errors (e.g. NRT_EXEC_UNIT_UNRECOVERABLE)
            # have been observed to clear on retry
            if attempt == 2:
                raise
            import time
            time.sleep(10)
    return np.concatenate(
        [_unstage_output(res.results[k]["out"]) for k in range(NCORES)],
        axis=0)
